# revision 11
# baseline (speedup 1.0000x reference)
"""Trainium2 Bass kernel for multi-head attention (B=2, S=2048, D=1024, H=16).

Sharding: 8 cores = 2 batches x 4 head-groups. Core c handles batch c//4 and
heads [4*(c%4), 4*(c%4)+4). Each core computes its 4 heads' Q/K/V projections
(column-sharded), attention, and a partial output projection over its 256
rows of Wo. Host sums the 4 partials per batch and adds the folded bias
(bo + bv @ Wo) there (tensor-parallel unshard).

v2 changes over the baseline:
  - A tunable subset of exp tiles is computed on the DVE as a one-
    instruction Schraudolph approximation (bf16 bit pattern via i16
    round of A*logit+B), freeing ACT (the exp engine) which is a
    co-bottleneck with the PE.
  - K-projection bias is dropped (exact: (q+bq)@k differs from
    (q+bq)@(k+bk) by a per-q constant, which softmax cancels); V bias
    and output bias fold into the host-side combine (attn weights sum
    to 1, so +bv maps to +bv@Wo on the output).
  - PSUM evacuation (K/V projections, Q bias add, output staging) is
    on the DVE; GPSIMD cannot read PSUM on this toolchain.
  - Projection matmul chains are interleaved so consecutive matmuls
    target different PSUM banks.
  - Output projection partials DMA straight from PSUM to DRAM.
"""

import os
import sys

import numpy as np

sys.path.insert(0, "/opt/trn_rl_repo")

import ml_dtypes

B, S, D, H = 2, 2048, 1024, 16
HD = D // H          # 64 head dim
NCORES = 8
CPB = 4              # cores per batch
NHC = 4              # heads per core
COLS = NHC * HD      # 256 projection columns per core
NG = 2               # groups of 128 cols (head pairs)
QTS = 512            # q tile size
NQT = S // QTS       # 4
KTS = 128            # k tile size
NKT = S // KTS       # 16
NDC = D // 128       # 8 contraction chunks for projections
DOT = 512            # out-proj column tile
NDO = D // DOT       # 2
SCALE = 1.0 / float(np.sqrt(HD))

# Schraudolph exp-on-DVE: bf16 bits of exp(SCALE*x) ~ i16(A*x + B)
EXP_A = SCALE * (2.0 ** 7) / float(np.log(2.0))
EXP_B = 16256.0 - 7.25

# which (c, ph) exp tiles go to the DVE (per q tile); rest go to ACT
DVE_TILES = frozenset((c, ph) for c in range(NKT) for ph in range(NG)
                      if c % 4 == 2)

_PROGRAMS = {}


def _build_program(loopn=1):
    import concourse.bass as bass
    import concourse.tile as tile
    from concourse import bacc
    import concourse.mybir as mybir

    f32 = mybir.dt.float32
    bf16 = mybir.dt.bfloat16
    i16 = mybir.dt.int16
    AF = mybir.ActivationFunctionType
    OP = mybir.AluOpType
    PSUM = bass.MemorySpace.PSUM

    nc = bacc.Bacc("TRN2", target_bir_lowering=False, debug=False)

    qT_d = nc.dram_tensor("qT", [D, S], bf16, kind="ExternalInput")
    kT_d = nc.dram_tensor("kT", [D, S], bf16, kind="ExternalInput")
    vT_d = nc.dram_tensor("vT", [D, S], bf16, kind="ExternalInput")
    wq_d = nc.dram_tensor("wq", [D, COLS], bf16, kind="ExternalInput")
    wk_d = nc.dram_tensor("wk", [D, COLS], bf16, kind="ExternalInput")
    wv_d = nc.dram_tensor("wv", [D, COLS], bf16, kind="ExternalInput")
    wo_d = nc.dram_tensor("wo", [COLS, D], bf16, kind="ExternalInput")
    bqr_d = nc.dram_tensor("bqr", [128, NG], f32, kind="ExternalInput")
    out_d = nc.dram_tensor("out", [S, D], bf16, kind="ExternalOutput")

    with tile.TileContext(nc) as tc:
        with (
            tc.tile_pool(name="persist", bufs=1) as persist,
            tc.tile_pool(name="wpool", bufs=1) as wpool,
            tc.tile_pool(name="xstream", bufs=2) as xstream,
            tc.tile_pool(name="rpool", bufs=2) as rpool,
            tc.tile_pool(name="expp", bufs=1) as expp,
            tc.tile_pool(name="lpp", bufs=2, space=PSUM) as lpp,
            tc.tile_pool(name="avp", bufs=1, space=PSUM) as avp,
            tc.tile_pool(name="dpp", bufs=1, space=PSUM) as dpp,
        ):
            # ---- persistent SBUF tiles ----
            QhT = persist.tile([128, NG, S], bf16)       # [p, grp, s]
            KhT = persist.tile([128, NG, S], bf16)
            Vh = persist.tile([128, NKT, COLS], bf16)    # [p, ktile, col]
            attnT = persist.tile([128, NG, S], bf16)
            wo_sb = persist.tile([128, NG, D], bf16)
            bqr_sb = persist.tile([128, NG], f32)
            ones64 = persist.tile([128, 64], bf16)       # dn stationary

            warm_sb = persist.tile([128, 1], f32)
            nc.vector.memset(ones64[:], 1.0)
            nc.vector.memset(warm_sb[:], 0.0)
            # hoist the ACT exp-table load to t=0 (overlaps the input DMAs)
            nc.scalar.activation(warm_sb[:], warm_sb[:], AF.Exp)
            nc.sync.dma_start(out=bqr_sb[:], in_=bqr_d[:])

            def body(_iv=None):
                wq_sb = wpool.tile([128, NDC, COLS], bf16, tag="wq", name="wq_sb")
                wk_sb = wpool.tile([128, NDC, COLS], bf16, tag="wk", name="wk_sb")
                wv_sb = wpool.tile([128, NDC, COLS], bf16, tag="wv", name="wv_sb")
                nc.sync.dma_start(out=wq_sb[:],
                                  in_=wq_d[:].rearrange("(c p) n -> p c n", p=128))

                # Q and K projections -> transposed head layout, streamed by
                # q tile; the two head-group chains are interleaved so
                # consecutive matmuls hit different PSUM banks
                qT_r = qT_d[:].rearrange("(c p) (t n) -> p c t n", p=128, n=QTS)
                kT_r = kT_d[:].rearrange("(c p) (t n) -> p c t n", p=128, n=QTS)
                for dst, w_sb, x_r, xtag in (
                    (QhT, wq_sb, qT_r, "qx"),
                    (KhT, wk_sb, kT_r, "kx"),
                ):
                    for qt in range(NQT):
                        x_sb = xstream.tile([128, NDC, QTS], bf16, tag=xtag,
                                            name="x_sb")
                        nc.sync.dma_start(out=x_sb[:], in_=x_r[:, :, qt, :])
                        ps = [avp.tile([128, QTS], f32, tag=f"av{g}",
                                       name="qk_ps") for g in range(NG)]
                        for dc in range(NDC):
                            for g in range(NG):
                                nc.tensor.matmul(
                                    ps[g][:],
                                    w_sb[:, dc, g * 128:(g + 1) * 128],
                                    x_sb[:, dc, :],
                                    start=(dc == 0), stop=(dc == NDC - 1),
                                )
                        for g in range(NG):
                            sl = dst[:, g, qt * QTS:(qt + 1) * QTS]
                            if dst is QhT:
                                nc.vector.tensor_scalar_add(
                                    sl, ps[g][:], bqr_sb[:, g:g + 1])
                            else:
                                nc.vector.tensor_copy(sl, ps[g][:])
                    if dst is QhT:
                        # K weights load after the q stream is underway
                        nc.sync.dma_start(
                            out=wk_sb[:],
                            in_=wk_d[:].rearrange("(c p) n -> p c n", p=128))

                nc.sync.dma_start(out=wv_sb[:],
                                  in_=wv_d[:].rearrange("(c p) n -> p c n", p=128))
                nc.sync.dma_start(out=wo_sb[:],
                                  in_=wo_d[:].rearrange("(c p) d -> p c d", p=128))

                # V projection -> natural [s, col] layout (overlaps attention)
                vT_r = vT_d[:].rearrange("(c p) (t n) -> p c t n", p=128, n=QTS)
                for vt in range(NQT):
                    v_sb = xstream.tile([128, NDC, QTS], bf16, tag="vx",
                                        name="v_sb")
                    nc.sync.dma_start(out=v_sb[:], in_=vT_r[:, :, vt, :])
                    v_ps = [dpp.tile([128, COLS], f32, tag=f"dn{sst % 2}",
                                     name="v_ps") for sst in range(2)]
                    for half in range(2):
                        for dc in range(NDC):
                            for sst in range(2):
                                s4 = half * 2 + sst
                                nc.tensor.matmul(
                                    v_ps[sst][:],
                                    v_sb[:, dc, s4 * 128:(s4 + 1) * 128],
                                    wv_sb[:, dc, :],
                                    start=(dc == 0), stop=(dc == NDC - 1),
                                )
                        for sst in range(2):
                            st = vt * 4 + half * 2 + sst
                            nc.vector.tensor_copy(Vh[:, st, :], v_ps[sst][:])
                        if half == 0:
                            v_ps = [dpp.tile([128, COLS], f32, tag=f"dn{s % 2}",
                                             name="v_ps") for s in range(2)]

                def emit_avdn(c, expT, av_t, dn_t):
                    for ph in range(NG):
                        for h2 in range(2):
                            h = 2 * ph + h2
                            nc.tensor.matmul(
                                av_t[ph][h2 * 64:(h2 + 1) * 64, :],
                                Vh[:, c, h * HD:(h + 1) * HD],
                                expT[:, h, c, :],
                                start=(c == 0), stop=(c == NKT - 1),
                                tile_position=(0, h2 * 64),
                                skip_group_check=True,
                            )
                        for h2 in range(2):
                            h = 2 * ph + h2
                            nc.tensor.matmul(
                                dn_t[ph][h2 * 64:(h2 + 1) * 64, :],
                                ones64[:, :],
                                expT[:, h, c, :],
                                start=(c == 0), stop=(c == NKT - 1),
                                tile_position=(0, h2 * 64),
                                skip_group_check=True,
                            )

                for qt in range(NQT):
                    q0 = qt * QTS
                    expT = expp.tile([128, NHC, NKT, QTS], bf16, tag="expT",
                                     name="expT")
                    expTi = expT[:].bitcast(i16)
                    av_t = [avp.tile([128, QTS], f32, tag=f"av{ph}",
                                     name=f"av{ph}") for ph in range(NG)]
                    dn_t = [dpp.tile([128, QTS], f32, tag=f"dn{ph}",
                                     name=f"dn{ph}") for ph in range(NG)]

                    # logits + exp per (pair, ktile); av/dn delayed one ktile
                    # so the PE always has ready work while ACT/DVE run exp
                    for c in range(NKT):
                        for ph in range(NG):
                            lp = lpp.tile([128, 2, QTS], f32, tag="Lp", name="lp")
                            for h2 in range(2):
                                pb = h2 * 64
                                nc.tensor.matmul(
                                    lp[:, h2, :],
                                    KhT[pb:pb + 64, ph, c * 128:(c + 1) * 128],
                                    QhT[pb:pb + 64, ph, q0:q0 + QTS],
                                    start=True, stop=True,
                                    tile_position=(pb, 0),
                                )
                            if (c, ph) in DVE_TILES:
                                nc.vector.tensor_scalar(
                                    expTi[:, 2 * ph:2 * ph + 2, c, :],
                                    lp[:],
                                    EXP_A, EXP_B,
                                    OP.mult, OP.add,
                                )
                            else:
                                nc.scalar.activation(
                                    expT[:, 2 * ph:2 * ph + 2, c, :],
                                    lp[:],
                                    AF.Exp, scale=SCALE,
                                )
                        if c > 0:
                            emit_avdn(c - 1, expT, av_t, dn_t)
                    emit_avdn(NKT - 1, expT, av_t, dn_t)

                    # normalize: attnT = av / denom (denoms row-replicated)
                    for ph in range(NG):
                        rb_t = rpool.tile([128, QTS], f32, tag="rb", name="rb_t")
                        nc.vector.reciprocal(rb_t[:], dn_t[ph][:])
                        nc.vector.tensor_mul(attnT[:, ph, q0:q0 + QTS],
                                             av_t[ph][:], rb_t[:])

                    # output projection for this q tile (partial, 256 rows);
                    # no bias (host adds bo + bv@Wo); bf16 partials staged
                    # through SBUF on GPSIMD, summed in f32 on the host
                    for qs in range(QTS // 128):
                        r0 = q0 + qs * 128
                        op_ps = [dpp.tile([128, DOT], f32, tag=f"dn{do}",
                                          name="op_ps") for do in range(NDO)]
                        for ch in range(NG):
                            for do in range(NDO):
                                nc.tensor.matmul(
                                    op_ps[do][:],
                                    attnT[:, ch, r0:r0 + 128],
                                    wo_sb[:, ch, do * DOT:(do + 1) * DOT],
                                    start=(ch == 0), stop=(ch == NG - 1),
                                )
                        for do in range(NDO):
                            st_t = rpool.tile([128, DOT], bf16, tag=f"st{do}",
                                              name="st_t")
                            nc.vector.tensor_copy(st_t[:], op_ps[do][:])
                            nc.sync.dma_start(
                                out=out_d[r0:r0 + 128, do * DOT:(do + 1) * DOT],
                                in_=st_t[:])

            if loopn == 1:
                body()
            else:
                with tc.For_i(0, loopn, 1) as iv:
                    body(iv)

    nc.compile()
    return nc


def _get_program(loopn=1):
    if loopn not in _PROGRAMS:
        _PROGRAMS[loopn] = _build_program(loopn)
    return _PROGRAMS[loopn]


def make_in_maps(q, k, v, Wq, Wk, Wv, Wo, bq, bk, bv, bo):
    bf = ml_dtypes.bfloat16
    q = np.asarray(q, np.float32)
    k = np.asarray(k, np.float32)
    v = np.asarray(v, np.float32)
    Wq = np.asarray(Wq, np.float32)
    Wk = np.asarray(Wk, np.float32)
    Wv = np.asarray(Wv, np.float32)
    Wo = np.asarray(Wo, np.float32)
    bq = np.asarray(bq, np.float32)
    bv = np.asarray(bv, np.float32)
    bo = np.asarray(bo, np.float32)

    qT = [np.ascontiguousarray(q[b].T).astype(bf) for b in range(B)]
    kT = [np.ascontiguousarray(k[b].T).astype(bf) for b in range(B)]
    vT = [np.ascontiguousarray(v[b].T).astype(bf) for b in range(B)]

    in_maps = []
    for c in range(NCORES):
        b, g = divmod(c, CPB)
        cs = slice(g * COLS, (g + 1) * COLS)
        in_maps.append({
            "qT": qT[b],
            "kT": kT[b],
            "vT": vT[b],
            "wq": np.ascontiguousarray(Wq[:, cs]).astype(bf),
            "wk": np.ascontiguousarray(Wk[:, cs]).astype(bf),
            "wv": np.ascontiguousarray(Wv[:, cs]).astype(bf),
            "wo": np.ascontiguousarray(Wo[cs, :]).astype(bf),
            "bqr": np.ascontiguousarray(bq[cs].reshape(NG, 128).T),
        })
    return in_maps


def host_bias(Wo, bv, bo):
    """Per-batch output bias folded on the host: bo + bv @ Wo."""
    return (np.asarray(bo, np.float32)
            + np.asarray(bv, np.float32) @ np.asarray(Wo, np.float32))


def combine_outputs(results, bias):
    out = np.zeros((B, S, D), np.float32)
    for c in range(NCORES):
        out[c // CPB] += np.asarray(results[c]["out"], np.float32)
    out += bias.reshape(1, 1, D)
    return out


def kernel(q, k, v, Wq, Wk, Wv, Wo, bq, bk, bv, bo):
    from concourse.bass_utils import run_bass_kernel_spmd

    nc = _get_program()
    in_maps = make_in_maps(q, k, v, Wq, Wk, Wv, Wo, bq, bk, bv, bo)
    res = run_bass_kernel_spmd(nc, in_maps, list(range(NCORES)))
    return combine_outputs(res.results, host_bias(Wo, bv, bo))
